# revision 12
# baseline (speedup 1.0000x reference)
"""Multi-head attention (16 heads, RoPE, causal) for Trainium2, 8 NeuronCores.

Sharding: data-parallel over batch (2) x tensor-parallel over head groups (4),
one (batch, head-group-of-4) pair per core. Each core computes its 4 heads'
attention feature-major (transposed) and a partial output projection
outT = Wo_slice^T @ Y^T [1024, 2048] in bf16; the host sums the 4 partials
per batch in fp32 and transposes back.

v2 design notes (on top of the v1 feature-major layout):
  - Startup: small consts land first, ~64 warmup matmuls keep the PE busy so
    the HAM clock-gate opens before real work; x streams in column halves so
    the first Q/K projection groups complete as soon as the left half lands.
  - Attention processes heads in PAIRS (even head on PE rows 0-63, odd head
    on rows 64-127). The K=64 S^T matmuls of the two heads land in different
    row-groups of the PE array (tile_position (0,0)/(64,0) auto-derived) and
    execute concurrently.
  - exp() split: even head exps on the scalar engine (exact), odd head on the
    vector engine via a custom DVE op computing (1+x/32)^32 (softmax-invariant
    log-space error <= |s|^2/64; a host-side exact Cauchy-Schwarz bound on
    max|logit| gates this path). The DVE variant fuses the causal mask of
    diagonal tiles via precomputed [128,1024] mask tiles.
  - Diagonal-tile causal masking for the scalar-exp head runs on the (idle)
    GPSIMD engine.
  - One unified 3-deep [128,1024] PSUM pool for S^T/projection/out-proj psum:
    the 3rd buffer breaks the S^T -> exp write-after-read chain.
  - Out-projection evacuation (PSUM -> bf16 SBUF) round-robins over
    vector/scalar/gpsimd; outT is bf16 (halves output DMA).
"""

import sys

sys.path.insert(0, "/opt/trn_rl_repo")
sys.path.insert(0, "/root/.axon_site")

import numpy as np

B, L, D = 2, 2048, 1024
H = 16                  # total heads
HD = 64                 # head dim
HPC = 4                 # heads per core
NCORES = 8
NT = 2                  # 128-row tiles per core of Q^T/K^T/Y^T (HPC*HD/128)
LC = L // 512           # 512-wide l chunks
KC = D // 128           # 128-deep contraction chunks over model dim
LT = L // 128           # 128-row l tiles

POLY_N = 32.0           # exp(x) ~= (1 + x/32)^32 on the DVE path
POLY_GATE = 0.7         # use the DVE poly path iff max|logit| <= this
EXP_GATE = 60.0         # no-max-subtraction exp overflow guard

_cache = {}
_ops_registered = []


def _register_polyexp():
    """Register the two custom DVE ops (plain + causal-masked) used for the
    odd-head exp. Idempotent; sha pins are derived on first compile."""
    if _ops_registered:
        return _ops_registered
    from concourse import dve_ops
    from concourse.dve_spec import Spec, Src0, Src1, C0, One, sq

    def _mk(name, body, ref):
        op = dve_ops.DveOp(name, Spec(body=body, reference=ref),
                           subdim=False, uops_sha={})
        dve_ops.OPS.append(op)
        dve_ops.CUSTOM_DVE_SPECS[name] = op.spec
        dve_ops._SUB_OPCODE_FOR_NAME[name] = (
            max(dve_ops._SUB_OPCODE_FOR_NAME.values()) + 1)
        for ver in ("v3", "v4"):
            try:
                op.compile(ver)
            except ValueError as e:
                import re
                m = re.search(r'="([0-9a-f]+)"', str(e))
                if not m:
                    raise
                op = dve_ops.DveOp(name, op.spec, subdim=False,
                                   uops_sha={**op.uops_sha, ver: m.group(1)})
            except KeyError:
                pass  # ver table absent for this trn gen
        dve_ops.OPS[-1] = op
        dve_ops.CUSTOM_DVE_SPECS[name] = op.spec
        return op

    p32 = sq(sq(sq(sq(sq(Src0 * C0 + One)))))

    def ref32(in0, in1, s0, s1, imm2):
        return ((in0.astype(np.float32) * np.float32(s0) + np.float32(1.0))
                ** 32).astype(np.float32)

    def ref32m(in0, in1, s0, s1, imm2):
        return (((in0.astype(np.float32) * np.float32(s0) + np.float32(1.0))
                 ** 32) * in1).astype(np.float32)

    op_plain = _mk("POLYEXP32_ANT", p32, ref32)
    op_mask = _mk("POLYEXP32M_ANT", p32 * Src1, ref32m)
    _ops_registered.extend([op_plain, op_mask])
    return _ops_registered


def _build_nc(causal: bool, poly: bool, pair_heads: bool = True,
              recip_psum: bool = False):
    # recip_psum=True is broken on HW: the reciprocal_approx_fast BITWISE_NOT
    # bit-trick reads garbage through the DVE PSUM read path (CoreSim passes).
    import contextlib

    import concourse.bass as bass
    import concourse.tile as tile
    from concourse import bacc, mybir

    F32 = mybir.dt.float32
    BF16 = mybir.dt.bfloat16
    EXP = mybir.ActivationFunctionType.Exp
    if poly:
        op_plain, op_mask = _register_polyexp()

    nc = bacc.Bacc("TRN2", target_bir_lowering=False, debug=False, num_devices=NCORES)

    xT = nc.dram_tensor("xT", [D, L], BF16, kind="ExternalInput")
    wq = nc.dram_tensor("wq", [D, 256], BF16, kind="ExternalInput")
    wk = nc.dram_tensor("wk", [D, 256], BF16, kind="ExternalInput")
    wv = nc.dram_tensor("wv", [D, 256], BF16, kind="ExternalInput")
    wo = nc.dram_tensor("wo", [256, D], BF16, kind="ExternalInput")
    cos128 = nc.dram_tensor("cos128", [128, L], BF16, kind="ExternalInput")
    srot128 = nc.dram_tensor("srot128", [128, L], BF16, kind="ExternalInput")
    mk4 = nc.dram_tensor("mk4", [128, 128], BF16, kind="ExternalInput")
    outT = nc.dram_tensor("outT", [D, L], BF16, kind="ExternalOutput")

    with tile.TileContext(nc) as tc, \
         nc.allow_low_precision(reason="bf16 matmul pipeline by design"), \
         contextlib.ExitStack() as ctx:
        p_w = ctx.enter_context(tc.tile_pool(name="p_w", bufs=24))
        p_wo = ctx.enter_context(tc.tile_pool(name="p_wo", bufs=2))
        p_const = ctx.enter_context(tc.tile_pool(name="p_const", bufs=6))
        p_xt = ctx.enter_context(tc.tile_pool(name="p_xt", bufs=8))
        p_qt = ctx.enter_context(tc.tile_pool(name="p_qt", bufs=2))
        p_kt = ctx.enter_context(tc.tile_pool(name="p_kt", bufs=2))
        p_yt = ctx.enter_context(tc.tile_pool(name="p_yt", bufs=2))
        p_v = ctx.enter_context(tc.tile_pool(name="p_v", bufs=16))
        p_pt = ctx.enter_context(tc.tile_pool(name="p_pt", bufs=10))
        p_tmp = ctx.enter_context(tc.tile_pool(name="p_tmp", bufs=3))
        p_z = ctx.enter_context(tc.tile_pool(name="p_z", bufs=3))
        p_oc = ctx.enter_context(tc.tile_pool(name="p_oc", bufs=6))
        # unified big PSUM pool: 3 x [128,1024] fp32 (6 banks); used for
        # warmup, Q/K/V projections, S^T tiles, and out-projection psum
        pbig = ctx.enter_context(tc.tile_pool(name="pbig", bufs=3, space="PSUM"))
        pso = ctx.enter_context(tc.tile_pool(name="pso", bufs=2, space="PSUM"))

        # ---- DMA issue order (Sync queue is FIFO; priority first) ----
        mk_t = p_const.tile([128, 128], BF16, tag="tri")
        nc.sync.dma_start(out=mk_t, in_=mk4.ap())
        wq_sb, wk_sb, wv_sb, x_sb = [], [], [], []
        for kc in range(KC):
            for srct, dst in ((wq, wq_sb), (wk, wk_sb)):
                w_t = p_w.tile([128, 256], BF16, tag="w")
                nc.sync.dma_start(out=w_t, in_=srct.ap()[kc * 128:(kc + 1) * 128, :])
                dst.append(w_t)
        cos_t = p_const.tile([128, L], BF16, tag="const")
        srot_t = p_const.tile([128, L], BF16, tag="const")
        nc.sync.dma_start(out=cos_t[:, 0:1024], in_=cos128.ap()[:, 0:1024])
        nc.sync.dma_start(out=srot_t[:, 0:1024], in_=srot128.ap()[:, 0:1024])
        for kc in range(KC):
            x_t = p_xt.tile([128, L], BF16, tag="xt", name=f"xt{kc}")
            nc.sync.dma_start(out=x_t[:, 0:1024],
                              in_=xT.ap()[kc * 128:(kc + 1) * 128, 0:1024])
            x_sb.append(x_t)
        for kc in range(KC):
            w_t = p_w.tile([128, 256], BF16, tag="w")
            nc.sync.dma_start(out=w_t, in_=wv.ap()[kc * 128:(kc + 1) * 128, :])
            wv_sb.append(w_t)
        wo_sb = []
        for kc2 in range(2):
            wo_t = p_wo.tile([128, D], BF16, tag="wo")
            nc.sync.dma_start(out=wo_t, in_=wo.ap()[kc2 * 128:(kc2 + 1) * 128, :])
            wo_sb.append(wo_t)
        nc.sync.dma_start(out=cos_t[:, 1024:2048], in_=cos128.ap()[:, 1024:2048])
        nc.sync.dma_start(out=srot_t[:, 1024:2048], in_=srot128.ap()[:, 1024:2048])
        for kc in range(KC):
            nc.sync.dma_start(out=x_sb[kc][:, 1024:2048],
                              in_=xT.ap()[kc * 128:(kc + 1) * 128, 1024:2048])

        # ---- PE warmup: release the HAM clock throttle before real work ----
        for i in range(40):
            wps = pbig.tile([128, 128], F32, tag="big")
            nc.tensor.matmul(wps[:, :], mk_t[:, :], mk_t[:, :], start=True, stop=True)
        for i in range(24):
            wps = pbig.tile([128, 256], F32, tag="big")
            nc.tensor.matmul(wps[:, :], mk_t[:, :], wq_sb[0][:, :], start=True, stop=True)

        # ---- causal mask tiles for the DVE-poly path (built on chip) ----
        if causal and poly:
            m01 = p_const.tile([128, 1024], BF16, tag="msk")
            m23 = p_const.tile([128, 1024], BF16, tag="msk")
            nc.vector.memset(m01[:, :], 0.0)
            nc.vector.memset(m01[:, 128:512], 1.0)
            nc.vector.memset(m01[:, 768:1024], 1.0)
            nc.vector.tensor_copy(m01[:, 0:128], mk_t[:, :])
            nc.vector.tensor_copy(m01[:, 640:768], mk_t[:, :])
            nc.vector.memset(m23[:, :], 0.0)
            nc.vector.memset(m23[:, 384:512], 1.0)
            nc.vector.tensor_copy(m23[:, 256:384], mk_t[:, :])
            nc.vector.tensor_copy(m23[:, 896:1024], mk_t[:, :])

        qt_sb = [p_qt.tile([128, L], BF16, tag="qt", name=f"qt{i}") for i in range(NT)]
        kt_sb = [p_kt.tile([128, L], BF16, tag="kt", name=f"kt{i}") for i in range(NT)]
        yt_sb = [p_yt.tile([128, L], BF16, tag="yt", name=f"yt{i}") for i in range(NT)]
        v_sb = [p_v.tile([128, HPC, 65], BF16, tag="vaug", name=f"vaug{i}")
                for i in range(LT)]

        # ---- Q^T / K^T projections + RoPE ---------------------------
        def rope_evac(ps, trg, lc):
            sl = slice(lc * 512, (lc + 1) * 512)
            qraw = p_tmp.tile([128, 512], BF16, tag="qraw")
            nc.scalar.copy(qraw[:, :], ps[:, :])
            tmp = p_tmp.tile([128, 512], BF16, tag="tmp")
            for hh in range(2):
                b0 = hh * 64
                nc.vector.tensor_mul(tmp[b0:b0 + 32, :], qraw[b0 + 32:b0 + 64, :],
                                     srot_t[b0 + 32:b0 + 64, sl])
                nc.vector.tensor_mul(tmp[b0 + 32:b0 + 64, :], qraw[b0:b0 + 32, :],
                                     srot_t[b0:b0 + 32, sl])
            nc.vector.tensor_mul(trg[:, sl], qraw[:, :], cos_t[:, sl])
            nc.vector.tensor_add(trg[:, sl], trg[:, sl], tmp[:, :])

        def proj_group(w_list, trg_list, nt, lc0):
            # weight tile loaded into the PE once, reused for the lc pair
            ps_ab = [pbig.tile([128, 512], F32, tag="big", name=f"pj{i}")
                     for i in range(2)]
            for kc in range(KC):
                w_ap = w_list[kc][:, nt * 128:(nt + 1) * 128]
                for i in range(2):
                    nc.tensor.matmul(
                        ps_ab[i][:, :], w_ap,
                        x_sb[kc][:, (lc0 + i) * 512:(lc0 + i + 1) * 512],
                        start=(kc == 0), stop=(kc == KC - 1))
            for i in range(2):
                rope_evac(ps_ab[i][:, :], trg_list[nt], lc0 + i)

        def v_group(lt):
            ps = pbig.tile([128, 256], F32, tag="big")
            for kc in range(KC):
                nc.tensor.matmul(
                    ps[:, :], x_sb[kc][:, lt * 128:(lt + 1) * 128],
                    wv_sb[kc][:, :], start=(kc == 0), stop=(kc == KC - 1))
            va = v_sb[lt]
            nc.vector.memset(va[:, :, 64:65], 1.0)
            nc.vector.tensor_copy(
                va[:, :, 0:64], ps[:, :].rearrange("p (h v) -> p h v", h=HPC))

        # left half: projections for lc pair (0,1), V tiles 0..7
        for w_list, trg_list in ((wq_sb, qt_sb), (wk_sb, kt_sb)):
            for nt in range(NT):
                proj_group(w_list, trg_list, nt, 0)
        for lt in range(8):
            v_group(lt)

        # ---- attention -----------------------------------------------------
        evac_rr = [0]

        def attention_chunk(c):
            jmax = 4 * c + 3 if causal else LT - 1

            def trim(j):
                k = j - 4 * c
                return 128 * k if (causal and k >= 0) else 0

            for pair in range(2):
                nt = pair
                heads = (2 * pair, 2 * pair + 1)   # rows 0-63 / 64-127 of nt
                oaug = [pso.tile([65, 512], F32, tag="oaug", name=f"oa{hh}")
                        for hh in range(2)]
                lag = [[], []]

                def emit_o(hh, jp, pt):
                    h = heads[hh]
                    for s in range(2):
                        j = 2 * jp + s
                        t = trim(j)
                        nc.tensor.matmul(
                            oaug[hh][:, t:512], v_sb[j][:, h, :],
                            pt[:, s * 512 + t:(s + 1) * 512],
                            start=(j == 0), stop=(j == jmax))

                def head_st(hh, jp):
                    t0 = trim(2 * jp)
                    st = pbig.tile([128, 1024], F32, tag="big", name=f"st{hh}")
                    r0 = hh * 64
                    for s in range(2):
                        j = 2 * jp + s
                        # s=1 writes its full 512 window so the contiguous
                        # [t0:] region read by exp/poly is fully written
                        t = trim(j) if s == 0 else 0
                        nc.tensor.matmul(
                            st[:, s * 512 + t:(s + 1) * 512],
                            kt_sb[nt][r0:r0 + 64, j * 128:(j + 1) * 128],
                            qt_sb[nt][r0:r0 + 64, c * 512 + t:(c + 1) * 512],
                            start=True, stop=True)
                    return st

                def head_pt(hh, jp, st):
                    t0 = trim(2 * jp)
                    pt = p_pt.tile([128, 1024], BF16, tag="pt", name=f"pt{hh}")
                    if hh == 1 and poly:
                        diag = causal and jp >= 2 * c
                        if diag:
                            msk = m01 if jp == 2 * c else m23
                            nc.vector._custom_dve(
                                op_mask, out=pt[:, t0:], in0=st[:, t0:],
                                in1=msk[:, t0:], s0=1.0 / POLY_N)
                        else:
                            nc.vector._custom_dve(
                                op_plain, out=pt[:, :], in0=st[:, :],
                                s0=1.0 / POLY_N)
                    else:
                        nc.scalar.activation(pt[:, t0:], st[:, t0:], EXP)
                        if causal:
                            for s in range(2):
                                k = 2 * jp + s - 4 * c
                                if k >= 0:
                                    sl = slice(s * 512 + 128 * k,
                                               s * 512 + 128 * (k + 1))
                                    nc.gpsimd.tensor_mul(pt[:, sl], pt[:, sl],
                                                         mk_t[:, :])
                    lag[hh].append((jp, pt))
                    if len(lag[hh]) > 2:
                        jpo, pto = lag[hh].pop(0)
                        emit_o(hh, jpo, pto)

                if pair_heads:
                    for jp in range((jmax + 1) // 2):
                        t0 = trim(2 * jp)
                        sts = [pbig.tile([128, 1024], F32, tag="big",
                                         name=f"st{hh}") for hh in range(2)]
                        for s in range(2):
                            j = 2 * jp + s
                            t = trim(j) if s == 0 else 0
                            for hh in range(2):
                                r0 = hh * 64
                                nc.tensor.matmul(
                                    sts[hh][:, s * 512 + t:(s + 1) * 512],
                                    kt_sb[nt][r0:r0 + 64, j * 128:(j + 1) * 128],
                                    qt_sb[nt][r0:r0 + 64,
                                              c * 512 + t:(c + 1) * 512],
                                    start=True, stop=True)
                        head_pt(0, jp, sts[0])
                        head_pt(1, jp, sts[1])
                else:
                    for hh in range(2):
                        for jp in range((jmax + 1) // 2):
                            head_pt(hh, jp, head_st(hh, jp))
                for hh in range(2):
                    for jpo, pto in lag[hh]:
                        emit_o(hh, jpo, pto)
                # normalize: y = O / z, z from the ones-row (row 64)
                for hh in range(2):
                    r0 = hh * 64
                    csl = slice(c * 512, (c + 1) * 512)
                    zrow = p_z.tile([1, 512], F32, tag="zrow")
                    if recip_psum:
                        nc.vector.reciprocal_approx_fast(zrow[0:1, :],
                                                         oaug[hh][64:65, :])
                    else:
                        zs = p_z.tile([1, 512], F32, tag="zs")
                        nc.vector.tensor_copy(zs[0:1, :], oaug[hh][64:65, :])
                        nc.vector.reciprocal_approx_fast(zrow[0:1, :], zs[0:1, :])
                    zb = p_z.tile([64, 512], F32, tag="zb")
                    nc.gpsimd.partition_broadcast(zb[:, :], zrow[0:1, :])
                    nc.vector.tensor_mul(yt_sb[nt][r0:r0 + 64, csl],
                                         oaug[hh][0:64, :], zb[:, :])

            # ---- output projection for this chunk's columns ---------
            for ot in range(8):
                ps = pbig.tile([128, 512], F32, tag="big")
                for kc2 in range(2):
                    nc.tensor.matmul(
                        ps[:, :], wo_sb[kc2][:, ot * 128:(ot + 1) * 128],
                        yt_sb[kc2][:, c * 512:(c + 1) * 512],
                        start=(kc2 == 0), stop=(kc2 == 1))
                oc = p_oc.tile([128, 512], BF16, tag="oc")
                if evac_rr[0] % 2 == 0:
                    nc.vector.tensor_copy(oc[:, :], ps[:, :])
                else:
                    nc.scalar.copy(oc[:, :], ps[:, :])
                evac_rr[0] += 1
                nc.sync.dma_start(
                    out=outT.ap()[ot * 128:(ot + 1) * 128, c * 512:(c + 1) * 512],
                    in_=oc[:, :])

        attention_chunk(0)

        # right half: projections for lc pair (2,3), V tiles 8..15
        for w_list, trg_list in ((wq_sb, qt_sb), (wk_sb, kt_sb)):
            for nt in range(NT):
                proj_group(w_list, trg_list, nt, 2)
        for lt in range(8, LT):
            v_group(lt)

        for c in range(1, LC):
            attention_chunk(c)

    nc.compile()
    return nc


def _get_nc(causal: bool, poly: bool):
    key = (causal, poly)
    if key not in _cache:
        _cache[key] = _build_nc(causal, poly)
    return _cache[key]


def _rope_np(x):
    d, s = x.shape[-1], x.shape[-2]
    ts = np.arange(0, d, 2, dtype=np.float32)
    inv = 10000.0 ** (-ts / d)
    grid = np.arange(s, dtype=np.float32)[:, None] * inv[None, :]
    sin = np.repeat(np.sin(grid), 2, axis=-1)
    cos = np.repeat(np.cos(grid), 2, axis=-1)
    x1, x2 = x[..., ::2], x[..., 1::2]
    xs = np.stack([-x2, x1], axis=-1).reshape(x.shape)
    return x * cos + xs * sin


def _reference_np(x, mask, Wq, Wk, Wv, Wo):
    b, l, d = x.shape
    h, k_sz = H, D // H
    split = lambda t: t.reshape(b, l, h, k_sz).transpose(0, 2, 1, 3)
    q = split((x @ Wq) / np.sqrt(np.float32(d)))
    q = _rope_np(q)
    k = _rope_np(split(x @ Wk))
    v = split(x @ Wv)
    logits = np.einsum("bhik,bhjk->bhij", q, k) + mask
    m = logits.max(axis=-1, keepdims=True)
    p = np.exp(logits - m)
    a = p / p.sum(axis=-1, keepdims=True)
    y = np.einsum("bhij,bhjv->bhiv", a, v)
    y = y.transpose(0, 2, 1, 3).reshape(b, l, d)
    return (y @ Wo).astype(np.float32)


def _logit_bound(x, Wq, Wk):
    """Exact per-head Cauchy-Schwarz bound on max |q_i . k_j|. RoPE is a
    per-pair rotation, so per-head row norms are preserved."""
    qmax = np.zeros(H)
    kmax = np.zeros(H)
    for bi in range(x.shape[0]):
        q = (x[bi] @ Wq) / np.sqrt(np.float32(D))
        k = x[bi] @ Wk
        qn = np.sqrt((q.reshape(L, H, HD) ** 2).sum(axis=2)).max(axis=0)
        kn = np.sqrt((k.reshape(L, H, HD) ** 2).sum(axis=2)).max(axis=0)
        qmax = np.maximum(qmax, qn)
        kmax = np.maximum(kmax, kn)
    return float((qmax * kmax).max())


def _host_consts():
    inv = 10000.0 ** (-np.arange(0, HD, 2, dtype=np.float32) / HD)
    grid = np.arange(L, dtype=np.float32)[None, :] * inv[:, None]   # [32, L]
    cos32 = np.cos(grid).astype(np.float32)
    sin32 = np.sin(grid).astype(np.float32)
    cos128 = np.ascontiguousarray(np.tile(cos32, (4, 1)))
    # srot rows r: +sin[r%32] for r%64 < 32, -sin[r%32] otherwise
    srot128 = np.ascontiguousarray(
        np.tile(np.concatenate([sin32, -sin32], axis=0), (2, 1)))
    tri = (np.arange(128)[None, :] >= np.arange(128)[:, None]).astype(np.float32)
    return cos128, srot128, np.ascontiguousarray(tri)


def _make_in_maps(x, Wq, Wk, Wv, Wo):
    import ml_dtypes
    bf16 = ml_dtypes.bfloat16

    cos128, srot128, mk4 = _host_consts()
    cos128 = cos128.astype(bf16)
    srot128 = srot128.astype(bf16)
    mk4 = mk4.astype(bf16)
    perm = np.concatenate([np.arange(0, 64, 2), np.arange(1, 64, 2)])
    Wq_s = (Wq / np.sqrt(np.float32(D))).astype(np.float32)
    in_maps = []
    for core in range(NCORES):
        bi, g = core // 4, core % 4
        xT_b = np.ascontiguousarray(x[bi].T.astype(bf16))
        wq_c = np.empty((D, 256), np.float32)
        wk_c = np.empty((D, 256), np.float32)
        for hh in range(HPC):
            h_abs = g * HPC + hh
            wq_c[:, hh * 64:(hh + 1) * 64] = Wq_s[:, h_abs * 64:(h_abs + 1) * 64][:, perm]
            wk_c[:, hh * 64:(hh + 1) * 64] = Wk[:, h_abs * 64:(h_abs + 1) * 64][:, perm]
        in_maps.append({
            "xT": xT_b,
            "wq": wq_c.astype(bf16),
            "wk": wk_c.astype(bf16),
            "wv": np.ascontiguousarray(Wv[:, g * 256:(g + 1) * 256].astype(bf16)),
            "wo": np.ascontiguousarray(Wo[g * 256:(g + 1) * 256, :].astype(bf16)),
            "cos128": cos128, "srot128": srot128, "mk4": mk4,
        })
    return in_maps


def kernel(x, mask, Wq, Wk, Wv, Wo):
    from concourse.bass_utils import run_bass_kernel_spmd

    x = np.asarray(x, dtype=np.float32)
    mask = np.asarray(mask, dtype=np.float32)
    Wq = np.asarray(Wq, dtype=np.float32)
    Wk = np.asarray(Wk, dtype=np.float32)
    Wv = np.asarray(Wv, dtype=np.float32)
    Wo = np.asarray(Wo, dtype=np.float32)

    # classify the mask
    m = mask.reshape(L, L)
    tril = np.tril(np.ones((L, L), dtype=bool))
    visible = m > -1e6
    if np.array_equal(visible, tril) and not m[tril].any():
        causal = True
    elif not m.any():
        causal = False
    else:
        return _reference_np(x, mask, Wq, Wk, Wv, Wo)

    bound = _logit_bound(x, Wq, Wk)
    if bound > EXP_GATE:
        return _reference_np(x, mask, Wq, Wk, Wv, Wo)
    poly = bound <= POLY_GATE

    in_maps = _make_in_maps(x, Wq, Wk, Wv, Wo)
    nc = _get_nc(causal, poly)
    res = run_bass_kernel_spmd(nc, in_maps, core_ids=list(range(NCORES)))

    out = np.empty((B, L, D), dtype=np.float32)
    for bi in range(B):
        acc = res.results[bi * 4]["outT"].astype(np.float32)
        for g in range(1, 4):
            acc += res.results[bi * 4 + g]["outT"].astype(np.float32)
        out[bi] = acc.T
    return out
